# revision 53
# baseline (speedup 1.0000x reference)
"""BitLinear forward (RMSNorm -> int8 activation quant -> ternary weight quant
-> matmul -> rescale) on 8 Trainium2 NeuronCores.

Sharding: data-parallel over rows. x (4,4096,1024) flattens to (16384,1024);
each core gets 2048 rows and the full weight (4096,1024). w_scale=mean|w| is
computed locally per core from a single pass over the full weight (the
per-shard approximation fails the tolerance; a collective AllReduce costs
~55us of latency, while the single-pass local sum is DMA-bandwidth-bound at
~50us anyway and needs no collective).

v2 schedule (single kernel, emission order == per-engine FIFO order):
 - scalar HWDGE ring: 32 w-strip loads (pass 1), then all output stores.
 - sync HWDGE ring: x tiles + re-loads of the strips not held in SBUF.
 - DVE: |w| abs-accum per strip (keeps ACT free for DMA triggers), x quant
   chains, ternarize clip ops, half the PSUM evacuations.
 - ACT: sqrt + RNE-scale ops of x quant, h1 ternarize cast, half the PSUM
   evacuations.
 - GPSIMD: partition all-reduce for w_scale, h0 ternarize cast, x ssq/xq.
 - PE: identity-matmul transposes + the 1024 main matmuls; strict queue
   order chosen so the PE never head-of-line blocks on not-yet-ready work
   (that blocking caused ~110us of PE idle + HAM re-throttle in v1).

Math notes:
 - x_q are exact integers in [-128,127] and w_t in {-1,0,1}; both are exact in
   bf16, so a bf16 matmul with fp32 PSUM accumulation reproduces the fp32
   reference einsum bit-for-bit (|sums| < 2^24).
 - round-half-to-even is done in fp32 via the magic constant 1.5*2^23.
 - ternary quantize sign(ws)*(|ws|>0.5) == RNE(clip(ws,-1,1)) exactly.
 - transposes to [k, r]/[k, n] layouts are identity matmuls (out = a.T @ I),
   batched 4 chunks per PSUM bank with one wide copy back to SBUF.
"""

import os

import numpy as np

import concourse.bass as bass
import concourse.mybir as mybir
import concourse.tile as tile
from concourse import bacc
from concourse.bass_utils import run_bass_kernel_spmd
from concourse.masks import make_identity
from concourse import bass_isa

F32 = mybir.dt.float32
BF16 = mybir.dt.bfloat16
ALU = mybir.AluOpType
AF = mybir.ActivationFunctionType

N_CORES = 8
R_FULL, K, N = 16384, 1024, 4096
R = R_FULL // N_CORES          # 2048 rows per core
RT = R // 128                  # 16 row tiles per core
KC = K // 128                  # 8 k-chunks
WS = N // 128                  # 32 weight strips (of 128 out-features)
NH = 2                         # n halves (2048 each)
S_HOLD = 8                     # strips kept resident between pass1 and burst

C_MAGIC = 12582912.0           # 1.5 * 2^23: fp32 round-to-nearest-even trick
Q_EPS = 1e-5
NORM_EPS = 1e-6


def build_nc(g_is_ones: bool):
    nc = bacc.Bacc("TRN2", target_bir_lowering=False)

    x_d = nc.dram_tensor("x", [R, K], F32, kind="ExternalInput")
    # w viewed as [16 batches, 2 strips, 128 rows, K] (row-major identical)
    w_d = nc.dram_tensor("w", [WS // 2, 2, 128, K], F32,
                         kind="ExternalInput")
    if not g_is_ones:
        g_d = nc.dram_tensor("g", [1, K], F32, kind="ExternalInput")
    out_d = nc.dram_tensor("out", [R, N], F32, kind="ExternalOutput")

    with tile.TileContext(nc) as tc:
        with (
            tc.tile_pool(name="persist", bufs=1) as persist,
            tc.tile_pool(name="xt", bufs=6) as xt_pool,
            tc.tile_pool(name="scr", bufs=2) as scr_pool,       # bf16 scratch
            tc.tile_pool(name="st", bufs=2) as st_pool,         # [128,1] stats
            tc.tile_pool(name="ux", bufs=1) as ux_pool,
            tc.tile_pool(name="xqp", bufs=2) as xq_pool,
            tc.tile_pool(name="xqT", bufs=16) as xqT_pool,
            tc.tile_pool(name="w1b", bufs=4) as w1b_pool,       # w pass1 batches
            tc.tile_pool(name="rs", bufs=3) as rs_pool,         # re-stream strips
            tc.tile_pool(name="uv", bufs=2) as uv_pool,
            tc.tile_pool(name="wtn", bufs=2) as wtn_pool,
            tc.tile_pool(name="stg", bufs=2) as stage_pool,
            tc.tile_pool(name="csp", bufs=16) as cs_pool,
            tc.tile_pool(name="pmm", bufs=8, space="PSUM") as psum_mm,
        ):
            # ---- constants ----
            cb = persist.tile([128, 1], F32, tag="cb")
            nc.vector.memset(cb[:], C_MAGIC)

            if not g_is_ones:
                g_row = persist.tile([1, K], F32, tag="g_row")
                nc.sync.dma_start(g_row[:], g_d[:])
                g_b = persist.tile([128, K], F32, tag="g_b")
                nc.gpsimd.partition_broadcast(g_b[:], g_row[0:1, :])

            wTT = [
                persist.tile([128, KC, N // NH], BF16, tag=f"wTT{h}",
                             name=f"wTT{h}")
                for h in range(NH)
            ]
            wpart = persist.tile([128, WS], F32, tag="wpart")
            wall = persist.tile([128, WS], F32, tag="wall")
            wsb = persist.tile([128, 1], F32, tag="wsb")
            invb = persist.tile([128, 1], F32, tag="invb")

            xqT_tiles = {}
            cs_tiles = {}
            xsc_tiles = {}

            # ================= emission helpers =================

            # transposes use the DMA XBAR (SBUF->SBUF, bf16): it produces
            # exactly the chunk-major [k%128, k//128, col] layout the mm
            # consumes, takes zero PE/DVE/ACT time, and frees all PSUM
            # banks for the matmuls

            def emit_x_load(t, eng):
                xt = xt_pool.tile([128, K], F32, tag="xt", name=f"xt{t}")
                eng.dma_start(xt[:], x_d[t * 128:(t + 1) * 128, :])
                return xt

            def emit_x_quant(t, xt, late):
                """RMSNorm stats + int8 quant -> xq bf16; transpose emitted
                separately. late=True routes ssq/xq to gpsimd."""
                with nc.named_scope("x_quant"):
                    if g_is_ones:
                        xg = xt
                    else:
                        xg = xt_pool.tile([128, K], F32, tag="xg",
                                          name=f"xg{t}")
                        nc.vector.tensor_mul(xg[:], xt[:], g_b[:])

                    xsq = scr_pool.tile([128, K], BF16, tag="xsq",
                                        name=f"xsq{t}")
                    ssq = st_pool.tile([128, 1], F32, tag="ssq")
                    nc.vector.scalar_tensor_tensor(
                        out=xsq[:], in0=xt[:], scalar=1.0, in1=xt[:],
                        op0=ALU.mult, op1=ALU.mult, accum_out=ssq[:])
                    am = st_pool.tile([128, 1], F32, tag="am")
                    nc.vector.tensor_reduce(
                        am[:], xg[:], axis=mybir.AxisListType.X, op=ALU.max,
                        apply_absolute_value=True)

                    # rs = 1/sqrt(ms + eps) with one Newton step on sqrt
                    ms = st_pool.tile([128, 1], F32, tag="ms")
                    nc.vector.tensor_scalar(
                        out=ms[:], in0=ssq[:], scalar1=1.0 / K,
                        scalar2=NORM_EPS, op0=ALU.mult, op1=ALU.add)
                    s0 = st_pool.tile([128, 1], F32, tag="s0")
                    nc.scalar.sqrt(s0[:], ms[:])
                    r0 = st_pool.tile([128, 1], F32, tag="r0")
                    nc.vector.reciprocal(r0[:], s0[:])
                    t0 = st_pool.tile([128, 1], F32, tag="t0")
                    nc.vector.tensor_mul(t0[:], ms[:], r0[:])
                    t1 = st_pool.tile([128, 1], F32, tag="t1")
                    nc.vector.tensor_add(t1[:], t0[:], s0[:])
                    s1 = st_pool.tile([128, 1], F32, tag="s1")
                    nc.vector.tensor_scalar(
                        out=s1[:], in0=t1[:], scalar1=0.5,
                        scalar2=None, op0=ALU.mult)
                    rs = st_pool.tile([128, 1], F32, tag="rs")
                    nc.vector.reciprocal(rs[:], s1[:])

                    axr = st_pool.tile([128, 1], F32, tag="axr")
                    nc.vector.tensor_mul(axr[:], am[:], rs[:])
                    xsc = st_pool.tile([128, 1], F32, tag="xsc",
                                       name=f"xsc{t}")
                    nc.vector.tensor_scalar(
                        out=xsc[:], in0=axr[:], scalar1=1.0 / 127.0,
                        scalar2=None, op0=ALU.mult)
                    xsc_tiles[t] = xsc
                    sx = st_pool.tile([128, 1], F32, tag="sx")
                    nc.vector.tensor_scalar(
                        out=sx[:], in0=axr[:], scalar1=1.0 / 127.0,
                        scalar2=Q_EPS, op0=ALU.mult, op1=ALU.add)
                    dx = st_pool.tile([128, 1], F32, tag="dx")
                    nc.vector.reciprocal(dx[:], sx[:])
                    srow = st_pool.tile([128, 1], F32, tag="srow")
                    nc.vector.tensor_mul(srow[:], rs[:], dx[:])

                    # x_q = RNE(xg * srow) via +C (ACT) then -C
                    ux = ux_pool.tile([128, K], F32, tag="ux", name=f"ux{t}")
                    nc.scalar.activation(
                        ux[:], xg[:], AF.Identity,
                        bias=cb[:, 0:1], scale=srow[:, 0:1])
                    xq = xq_pool.tile([128, K], BF16, tag="xq", name=f"xq{t}")
                    nc.vector.tensor_scalar(
                        out=xq[:], in0=ux[:], scalar1=C_MAGIC,
                        scalar2=None, op0=ALU.subtract)
                    return xq

            def emit_cs(t):
                cs = cs_pool.tile([128, 1], F32, tag="cs", name=f"cs{t}")
                nc.vector.tensor_mul(cs[:], xsc_tiles[t][:], wsb[:])
                cs_tiles[t] = cs

            def emit_x_tp(t, xq):
                xqT = xqT_pool.tile([128, KC, 128], BF16, tag="xqT",
                                    name=f"xqT{t}")
                eng = nc.sync if t % 2 == 0 else nc.scalar
                eng.dma_start(xqT[:], xq[:], transpose=True)
                xqT_tiles[t] = xqT

            def emit_w_load(b, eng):
                """Load batch b (2 strips) partition-major in one DMA."""
                dst = w1b_pool.tile([128, 2, K], F32, tag="w1b",
                                    name=f"w1b_{b}")
                eng.dma_start(dst[:], w_d[b].transpose([1, 0, 2]))
                return dst

            def emit_w_abs(s, src, eng):
                # abs+accum, split across ACT and DVE; emission is
                # interleaved with the batch triggers so a scalar-ring
                # trigger never waits on an ACT abs behind it
                wab = scr_pool.tile([128, K], BF16, tag="wab",
                                    name=f"wab{s}")
                if eng is nc.scalar:
                    nc.scalar.activation(wab[:], src, AF.Abs,
                                         accum_out=wpart[:, s:s + 1])
                else:
                    nc.vector.scalar_tensor_tensor(
                        out=wab[:], in0=src, scalar=-1.0, in1=src,
                        op0=ALU.mult, op1=ALU.max,
                        accum_out=wpart[:, s:s + 1])

            def emit_w_reload(s, eng):
                dst = rs_pool.tile([128, K], F32, tag="rs", name=f"wr{s}")
                eng.dma_start(dst[:], w_d[s // 2, s % 2])
                return dst

            wtn_tiles = {}

            def emit_tern_vec(s, src):
                """Clip+RNE strip to {-1,0,1} bf16 (DVE+ACT only)."""
                with nc.named_scope("w_ternarize"):
                    u = uv_pool.tile([128, K], F32, tag="uv", name=f"wu{s}")
                    nc.vector.tensor_scalar(
                        out=u[:], in0=src, scalar1=invb[:, 0:1],
                        scalar2=1.0, op0=ALU.mult, op1=ALU.min)
                    v = uv_pool.tile([128, K], F32, tag="uv", name=f"wv{s}")
                    nc.vector.tensor_scalar(
                        out=v[:], in0=u[:], scalar1=-1.0,
                        scalar2=C_MAGIC, op0=ALU.max, op1=ALU.add)
                    wtn = wtn_pool.tile([128, K], BF16, tag="wtn",
                                        name=f"wtn{s}")
                    nc.scalar.activation(wtn[:], v[:], AF.Copy,
                                         bias=-C_MAGIC)
                    wtn_tiles[s] = wtn

            def emit_w_tp(s):
                wtn = wtn_tiles.pop(s)
                h, hcol = s // (WS // NH), (s % (WS // NH)) * 128
                eng = nc.sync if s % 2 == 0 else nc.scalar
                eng.dma_start(wTT[h][:, :, hcol:hcol + 128], wtn[:],
                              transpose=True)

            def emit_mm_q(rt, qtr, gi=0):
                """Quarter-width mm group: 2 PSUM banks over cols
                [qtr*1024, (qtr+1)*1024); out store via gpsimd SWDGE."""
                xqT = xqT_tiles[rt]
                h, qc = qtr // 2, (qtr % 2) * 1024
                with nc.named_scope("mm"):
                    pst = [
                        psum_mm.tile([128, 512], F32, tag="pmm",
                                     name=f"pmm_{rt}_{qtr}_{q}")
                        for q in range(2)
                    ]
                    for j in range(KC):
                        for q in range(2):
                            nc.tensor.matmul(
                                pst[q][:],
                                lhsT=xqT[:, j, :],
                                rhs=wTT[h][:, j, qc + q * 512:
                                           qc + (q + 1) * 512],
                                start=(j == 0), stop=(j == KC - 1))
                with nc.named_scope("out_scale"):
                    cs = cs_tiles[rt]
                    stg = stage_pool.tile([128, 1024], F32, tag="stage",
                                          name=f"stg{rt}_{qtr}")
                    nc.scalar.activation(
                        stg[:, 0:512], pst[0][:], AF.Copy, scale=cs[:, 0:1])
                    nc.vector.tensor_scalar(
                        out=stg[:, 512:1024], in0=pst[1][:],
                        scalar1=cs[:, 0:1], scalar2=None, op0=ALU.mult)
                    # last few stores on the (idle) HWDGE rings: SWDGE has
                    # ~2us completion latency that would stretch the tail
                    if gi >= 56:
                        eng = nc.sync if gi % 2 == 0 else nc.scalar
                    else:
                        eng = nc.gpsimd
                    eng.dma_start(
                        out_d[rt * 128:(rt + 1) * 128,
                              qtr * 1024:(qtr + 1) * 1024],
                        stg[:])

            # ================= emission schedule =================

            # HWDGE rings (sync + scalar): x0/x1 first, then the 16 w
            # pass-1 batches, x2/x3, then the ternarize re-stream strips
            # with x4..15 woven in. Outputs go via gpsimd SWDGE so they
            # never queue behind these loads.
            xt_early = {}
            xt_early[0] = emit_x_load(0, nc.sync)
            xt_early[1] = emit_x_load(1, nc.scalar)
            # batch b on sync ring when b even (abs on DVE), scalar ring
            # when b odd (abs on ACT)
            w_src = []
            with nc.named_scope("w_abs_sum"):
                for b in range(4):
                    w_src.append(emit_w_load(b, nc.sync if b % 2 == 0
                                             else nc.scalar))
                for b in range(WS // 2):
                    eng = nc.vector if b % 2 == 0 else nc.scalar
                    emit_w_abs(2 * b, w_src[b][:, 0, :], eng)
                    emit_w_abs(2 * b + 1, w_src[b][:, 1, :], eng)
                    if b + 4 < WS // 2:
                        w_src.append(emit_w_load(
                            b + 4, nc.sync if b % 2 == 0 else nc.scalar))
            xt_early[2] = emit_x_load(2, nc.sync)
            xt_early[3] = emit_x_load(3, nc.scalar)

            # re-stream strips for ternarize (self-paced by rs pool);
            # weave x4..15 triggers so each lands before its quant chain
            # while only ever waiting on strictly-earlier vector ops
            burst_src = {}
            for s in range(6):
                burst_src[s] = emit_w_reload(
                    s, nc.sync if s % 2 == 0 else nc.scalar)[:]
            for t in (4, 5):
                xt_early[t] = emit_x_load(
                    t, nc.sync if t % 2 == 0 else nc.scalar)
            for s in (6, 7):
                burst_src[s] = emit_w_reload(
                    s, nc.sync if s % 2 == 0 else nc.scalar)[:]
            for t in (6, 7):
                xt_early[t] = emit_x_load(
                    t, nc.sync if t % 2 == 0 else nc.scalar)
            nxt = 8
            for s in range(8, WS):
                burst_src[s] = emit_w_reload(
                    s, nc.sync if s % 2 == 0 else nc.scalar)[:]
                if s >= 10 and nxt <= 15 and nxt <= s - 2:
                    xt_early[nxt] = emit_x_load(
                        nxt, nc.sync if nxt % 2 == 0 else nc.scalar)
                    nxt += 1

            # x0..3 quant + transpose run pre-invb (abs is on ACT; DVE is
            # free, and the PE gets early transpose work)
            for t in (0, 1, 2, 3):
                xq = emit_x_quant(t, xt_early[t], late=False)
                emit_x_tp(t, xq)

            # w_scale = mean|w|; inv = 1/(w_scale + eps)
            with nc.named_scope("w_scale"):
                nc.gpsimd.partition_all_reduce(
                    wall[:], wpart[:], channels=128,
                    reduce_op=bass_isa.ReduceOp.add)
                wsumb = st_pool.tile([128, 1], F32, tag="wsumb")
                nc.vector.reduce_sum(wsumb[:], wall[:],
                                     axis=mybir.AxisListType.X)
                nc.vector.tensor_scalar(
                    out=wsb[:], in0=wsumb[:], scalar1=1.0 / (N * K),
                    scalar2=None, op0=ALU.mult)
                speps = st_pool.tile([128, 1], F32, tag="speps")
                nc.vector.tensor_scalar(
                    out=speps[:], in0=wsumb[:], scalar1=1.0 / (N * K),
                    scalar2=Q_EPS, op0=ALU.mult, op1=ALU.add)
                nc.vector.reciprocal(invb[:], speps[:])
            for t in (0, 1, 2, 3):
                emit_cs(t)

            # ternarize burst: strips 0..7 only (they unblock the whole
            # q0 era); the other 24 strips are spread across mm groups
            for s in range(8):
                emit_tern_vec(s, burst_src[s])
                emit_w_tp(s)

            # ---- mm phase: 64 quarter-groups with interleaved work ----
            # each strip/tile's xbar transpose is emitted right after its
            # producer (it costs no engine time); each era's strips
            # complete before the era that consumes them
            chain_at = {t - 4: t for t in range(4, 16)}
            tern_at = {}
            for i in range(8):
                tern_at[1 + i] = 8 + i
                tern_at[17 + i] = 16 + i
                tern_at[33 + i] = 24 + i

            for qtr in range(4):
                for rt in range(RT):
                    gi = qtr * RT + rt
                    t_c = chain_at.get(gi)
                    if t_c is not None:
                        xq = emit_x_quant(t_c, xt_early[t_c], late=True)
                        emit_cs(t_c)
                        emit_x_tp(t_c, xq)
                    s_v = tern_at.get(gi)
                    if s_v is not None:
                        emit_tern_vec(s_v, burst_src[s_v])
                        emit_w_tp(s_v)
                    emit_mm_q(rt, qtr, gi)

    nc.compile()
    return nc


def _ensure_ntff_hook():
    """Make trace=True work: bass_utils imports antenv.axon_hooks, which is
    not present in this image. Shim it and install the ctypes-based NTFF
    profiling hook against libaxon_pjrt.so (same recipe as trn_boot)."""
    import sys
    import types
    try:
        import antenv.axon_hooks  # noqa: F401
        return
    except ImportError:
        pass
    mod = types.ModuleType("antenv.axon_hooks")
    mod._hook = None
    mod.set_axon_ntff_profile_hook = lambda h: setattr(mod, "_hook", h)
    mod.get_axon_ntff_profile_hook = lambda: mod._hook
    sys.modules["antenv.axon_hooks"] = mod
    import antenv
    antenv.axon_hooks = mod
    try:
        from trn_agent_boot.trn_boot import _ntff_profile_via_ctypes
        hook = _ntff_profile_via_ctypes("/opt/axon/libaxon_pjrt.so")
        if hook is not None:
            mod._hook = hook
    except Exception as e:  # degrade to no-trace
        print(f"ntff hook install failed: {e}")
    # no S3 in this sandbox; keep artifacts local
    import concourse.bass_utils as bu
    bu.upload_artifacts = lambda tmpdir: f"local://{tmpdir}"


_NC_CACHE = {}


def kernel(x: np.ndarray, weight: np.ndarray, norm_weight: np.ndarray) -> np.ndarray:
    x = np.ascontiguousarray(x, dtype=np.float32)
    weight = np.ascontiguousarray(weight, dtype=np.float32)
    norm_weight = np.ascontiguousarray(norm_weight, dtype=np.float32)

    B, S, Kin = x.shape
    xf = x.reshape(-1, Kin)
    g_is_ones = bool(np.all(norm_weight == 1.0))

    if g_is_ones not in _NC_CACHE:
        _NC_CACHE[g_is_ones] = build_nc(g_is_ones)
    nc = _NC_CACHE[g_is_ones]

    wq = weight.reshape(WS // 2, 2, 128, Kin)
    in_maps = []
    for i in range(N_CORES):
        m = {"x": xf[i * R:(i + 1) * R], "w": wq}
        if not g_is_ones:
            m["g"] = norm_weight.reshape(1, Kin)
        in_maps.append(m)

    trace = bool(int(os.environ.get("BITLIN_TRACE", "0")))
    if trace:
        _ensure_ntff_hook()
    res = run_bass_kernel_spmd(
        nc, in_maps, core_ids=list(range(N_CORES)), trace=trace,
    )
    if trace:
        kernel.last_results = res
    out = np.concatenate([r["out"] for r in res.results], axis=0)
    return out.reshape(B, S, weight.shape[0]).astype(np.float32)
